# revision 25
# baseline (speedup 1.0000x reference)
"""Multi-head causal attention (B=2, S=2048, D=1024, H=16) on 8 TRN2 NeuronCores.

Sharding: core c -> batch c//4, head-quarter c%4 (4 heads = 256 head dims).
Each core runs the full pipeline for its (batch, 4 heads); host sums the 4
row-sharded out-projection partials per batch + bias.

v2 design (PE-column-minimal, all matmuls bf16 at rate 1 col/cycle):
  - QKV projections: moving xT [128,512] bf16, 8-chunk contraction.
  - Scores: per 128-k-tile, live-q-trimmed moving qT (free 512-coff).
  - exp on Act into bf16 `at` tiles; causal diag masked by tri-mult on Pool.
  - PV in stationary-attention layout: out[q,65] += at[k,q].T @ [v|1][k,65]
    so each k-tile costs only 65 PE columns; 4 heads accumulate in ONE
    PSUM bank (start=True only on the very first matmul of the bank:
    pending-zero covers the whole bank, later head groups start=False).
  - rowsum rides along as the ones column; normalize = per-partition
    tensor_scalar on DVE, writing bf16 ctx; DMA-XBAR transpose to cT.
  - out-projection from cT bf16; PSUM->SBUF copies split Act/DVE; out DMA.
Emission is software-pipelined: PV(qs) blocks and out-proj(j-1) interleave
into the scores i-loop so PE keeps running while Act chews exps.
"""

import sys

import numpy as np

if "/opt/trn_rl_repo" not in sys.path:
    sys.path.insert(0, "/opt/trn_rl_repo")

import concourse.bass as bass
import concourse.mybir as mybir
import concourse.tile as tile
from concourse.bass import ts
from concourse.bass_utils import run_bass_kernel_spmd

P = 128          # partitions
S = 2048         # sequence length
DD = 1024        # model dim
DC = DD // P     # d-model chunks (8)
E = 256          # head dims per core (4 heads x 64)
H4 = 4           # heads per core
HD = 64
NQ = 4           # q chunks of 512
QC = 512
KT = S // P      # k tiles (16)
FD = 512         # out-proj free dim

F32 = mybir.dt.float32
BF16 = mybir.dt.bfloat16
EXP = mybir.ActivationFunctionType.Exp
MUL = mybir.AluOpType.mult


def _emit(tc, nc, xT_d, wq_d, wk_d, wv_d, wo_d, tri_d, out_d):
    with (
        tc.tile_pool(name="const", bufs=1) as const,
        tc.tile_pool(name="attn", bufs=34) as attn_pool,
        tc.tile_pool(name="small", bufs=4) as small,
        tc.tile_pool(name="ctxp", bufs=4) as ctxp,
        tc.tile_pool(name="ostage", bufs=4) as ostage,
        tc.tile_pool(name="pmm", bufs=2, space="PSUM") as pmm,
        tc.tile_pool(name="pacc", bufs=2, space="PSUM") as pacc,
        tc.tile_pool(name="psc", bufs=2, space="PSUM") as psc,
    ):
        xT = const.tile([P, DC, S], BF16)
        wq = const.tile([P, DC, E], BF16)
        wk = const.tile([P, DC, E], BF16)
        wv = const.tile([P, DC, E], BF16)
        wo = const.tile([P, 2, DD], BF16)
        tri = const.tile([P, P], BF16)
        qT = const.tile([P, 2, S], BF16)
        kT = const.tile([P, 2, S], BF16)
        vS = const.tile([P, KT, H4, HD + 1], BF16)
        cT = const.tile([P, 2, S], BF16)

        # j=0 slices of x^T first so the first projections can start early;
        # issue across both HWDGE queues (SP + Act) to halve serialization
        nc.sync.dma_start(wq[:, 0:4, :], wq_d[:, 0:4, :])
        nc.scalar.dma_start(wk[:, 0:4, :], wk_d[:, 0:4, :])
        nc.sync.dma_start(wq[:, 4:8, :], wq_d[:, 4:8, :])
        nc.scalar.dma_start(wk[:, 4:8, :], wk_d[:, 4:8, :])
        for c in range(0, DC, 2):
            eng = nc.sync if (c // 2) % 2 == 0 else nc.scalar
            eng.dma_start(xT[:, c:c + 2, 0:QC], xT_d[:, c:c + 2, 0:QC])
        nc.scalar.dma_start(wv[:], wv_d[:])
        nc.sync.dma_start(tri[:], tri_d[:])
        nc.sync.dma_start(xT[:, 0:4, QC:S], xT_d[:, 0:4, QC:S])
        nc.scalar.dma_start(xT[:, 4:8, QC:S], xT_d[:, 4:8, QC:S])
        nc.sync.dma_start(wo[:], wo_d[:])

        # ones column of [V|1] (rowsums of masked exp-scores come out of PV)
        nc.vector.memset(vS[:, :, :, HD], 1.0)

        def psum_copy(dst, src):
            # all PSUM->SBUF copies on VectorE; ScalarE stays exp-only
            nc.vector.tensor_copy(dst, src)

        def emit_qk_proj(j):
            for w_s, dst in ((wq, qT), (wk, kT)):
                for et in range(2):
                    ps = pmm.tile([P, QC], F32, tag="mm", name="ps_proj")
                    for c in range(DC):
                        nc.tensor.matmul(
                            ps[:],
                            lhsT=w_s[:, c, ts(et, P)],
                            rhs=xT[:, c, ts(j, QC)],
                            start=(c == 0),
                            stop=(c == DC - 1),
                        )
                    psum_copy(dst[:, et, ts(j, QC)], ps[:])

        def emit_v_proj(nt):
            psv = pmm.tile([P, E], F32, tag="mm", name="ps_v")
            for c in range(DC):
                nc.tensor.matmul(
                    psv[:],
                    lhsT=xT[:, c, ts(nt, P)],
                    rhs=wv[:, c, :],
                    start=(c == 0),
                    stop=(c == DC - 1),
                )
            psum_copy(
                vS[:, nt, :, 0:HD],
                psv[:].rearrange("p (h d) -> p h d", h=H4),
            )

        def emit_scores(j, i, at_tiles):
            # scores + exp for k-tile i against q-chunk j, both head pairs
            coff = max(0, P * (i - 4 * j))
            for hp in range(2):
                sc = psc.tile([P, 2, QC], F32, tag="sc", name="sc")
                at = attn_pool.tile([P, 2, QC], BF16, tag="at", name="at")
                for hh in range(2):
                    po = HD * hh
                    nc.tensor.matmul(
                        sc[:, hh, coff:QC],
                        lhsT=kT[po:po + HD, hp, ts(i, P)],
                        rhs=qT[po:po + HD, hp, j * QC + coff:(j + 1) * QC],
                        start=True,
                        stop=True,
                    )
                nc.scalar.activation(at[:, :, coff:QC], sc[:, :, coff:QC], EXP)
                if i >= 4 * j:  # diagonal 128x128 block: causal triangle
                    for hh in range(2):
                        nc.gpsimd.tensor_tensor(
                            at[:, hh, coff:coff + P],
                            at[:, hh, coff:coff + P],
                            tri[:],
                            MUL,
                        )
                at_tiles[i, hp] = at

        def emit_pv(j, qs, at_tiles, split=False):
            # ctx[q, d] for global q-subtile qs, all 4 heads in one PSUM bank.
            # split=True normalizes/transposes per head-pair (shorter drain
            # chain) - worth it only for the final subtile's tail.
            qo = P * (qs - 4 * j)
            pv = pacc.tile([P, H4, HD + 1], F32, tag="pv", name="pv")
            cq = ctxp.tile([P, H4, HD], BF16, tag="cq", name="cq")
            # explicit zero-init: hardware start_tensor_calc only overwrites
            # the bytes each matmul writes, so four head-groups sharing one
            # bank must accumulate (start=False) onto real zeros
            nc.vector.memset(pv[:], 0.0)

            def normalize(hp):
                lo = 2 * hp if split else 0
                hi = 2 * hp + 2 if split else H4
                rec = small.tile([P, hi - lo], F32, tag="rec", name="rec")
                nc.vector.reciprocal(rec[:], pv[:, lo:hi, HD])
                for h in range(lo, hi):
                    nc.vector.tensor_scalar(
                        cq[:, h, :], pv[:, h, 0:HD], rec[:, h - lo:h - lo + 1],
                        None, MUL,
                    )
                for half in range(hp, hp + 1) if split else range(2):
                    nc.sync.dma_start(
                        out=cT[:, half, ts(qs, P)],
                        in_=cq[:, 2 * half:2 * half + 2, :],
                        transpose=True,
                    )

            for hp in range(2):
                for hh in range(2):
                    h = 2 * hp + hh
                    for i in range(qs + 1):
                        nc.tensor.matmul(
                            pv[:, h, :],
                            lhsT=at_tiles[i, hp][:, hh, qo:qo + P],
                            rhs=vS[:, i, h, :],
                            start=False,
                            stop=(hh == 1 and i == qs),
                            skip_group_check=True,
                        )
                if split:
                    normalize(hp)
            if not split:
                normalize(0)

        def emit_out_proj(nt, fc, tail=False):
            po = pmm.tile([P, FD], F32, tag="mm", name="ps_out")
            for c in range(2):
                nc.tensor.matmul(
                    po[:],
                    lhsT=cT[:, c, ts(nt, P)],
                    rhs=wo[:, c, ts(fc, FD)],
                    start=(c == 0),
                    stop=(c == 1),
                )
            ob = ostage.tile([P, FD], F32, tag="ob", name="ob")
            if tail:  # Act copies (idle at the end) overlap SP DMA issues
                nc.scalar.copy(ob[:], po[:])
            else:
                psum_copy(ob[:], po[:])
            nc.sync.dma_start(out_d[ts(nt, P), ts(fc, FD)], ob[:])

        # Global software pipeline. The scores i-loop is Act(exp)-bound, so
        # every other PE unit (projection groups, PV blocks, out-proj tiles)
        # goes into a FIFO fill queue drained per-iteration by the Act-pace
        # surplus. Un-drained units carry across chunk boundaries, which
        # automatically defers PV/out-proj work into chunk 3's long
        # exp-only stretch.
        at_tiles = {}
        fill = []  # (cost_ns, kind, emit_fn)

        def drain(budget):
            while fill and budget > 0:
                cost, _, fn = fill.pop(0)
                fn()
                budget -= cost

        def force_drain(kind):
            # emit every queued unit up to and including the last of `kind`
            last = max((n for n, (_, k, _) in enumerate(fill) if k == kind),
                       default=-1)
            for _ in range(last + 1):
                _, _, fn = fill.pop(0)
                fn()

        deferred_out = []  # out-proj units hoarded for chunk 3's exp stretch
        in_last = [False]

        def push_pv(j, qs):
            cost = (qs + 1) * H4 * (HD + 1) * 0.42 + 600
            snap = dict(at_tiles)  # at_tiles mutates before deferred drain
            def go():
                emit_pv(j, qs, snap, split=(qs == 15))
                dst = fill if (in_last[0] or qs >= 12) else deferred_out
                tail = qs >= 14
                dst.append((450, "out", lambda: emit_out_proj(qs, 0, tail)))
                dst.append((450, "out", lambda: emit_out_proj(qs, 1, tail)))
            fill.append((cost, "pv", go))

        emit_qk_proj(0)
        for nt in range(4):
            emit_v_proj(nt)

        for j in range(NQ):
            if j + 1 < NQ:
                fill.append((1750, f"qk{j + 1}", lambda j1=j + 1: emit_qk_proj(j1)))
                for nt in range(4 * (j + 1), 4 * (j + 1) + 4):
                    fill.append((900, f"v{j + 1}", lambda nt=nt: emit_v_proj(nt)))
            if j > 0:
                force_drain(f"qk{j}")  # this chunk's q/k must exist
            if j == NQ - 1:
                in_last[0] = True
                fill.extend(deferred_out)
                deferred_out.clear()
            nk = 4 * (j + 1)
            for i in range(nk):
                if i == 4 * j and j > 0:
                    force_drain(f"v{j}")  # PV(j, 4j) is imminent
                emit_scores(j, i, at_tiles)
                # Act per iter: 2 exps of 2*(QC-coff) free each + overhead;
                # scores PE cost: 4 matmuls of (QC-coff) cols. Drain just
                # under the surplus so Act never waits on the next scores.
                w = QC - max(0, P * (i - 4 * j))
                drain(1.58 * w + 400)
                if i > 4 * j:
                    push_pv(j, i - 1)
            push_pv(j, 4 * j + 3)
        drain(float("inf"))


def _split_multi_waits(nc):
    """The TRN2 instruction encoding carries ONE sync-wait slot; this walrus
    build rejects instructions with more. Hoist extra waits onto standalone
    EventSemaphore instructions immediately before (same engine queue, same
    semantics)."""
    n = 0
    for f in nc.m.functions:
        for b in f.blocks:
            out = []
            for i in list(b.instructions):
                si = i.sync_info
                if si is not None and len(si.on_wait) > 1:
                    waits = list(si.on_wait)
                    for w in waits[:-1]:
                        n += 1
                        out.append(
                            mybir.InstEventSemaphore(
                                name=f"I-wsplit{n}",
                                engine=i.engine,
                                ins=[],
                                outs=[],
                                sync_info=mybir.SyncInfo(on_wait=[w], on_update=[]),
                            )
                        )
                    i.sync_info = mybir.SyncInfo(
                        on_wait=[waits[-1]], on_update=list(si.on_update)
                    )
                out.append(i)
            b.instructions = out


def build_nc(split_waits=True):
    nc = bass.Bass("TRN2", target_bir_lowering=False, debug=False)
    xT_d = nc.dram_tensor("xT", [P, DC, S], BF16, kind="ExternalInput").ap()
    wq_d = nc.dram_tensor("wqT", [P, DC, E], BF16, kind="ExternalInput").ap()
    wk_d = nc.dram_tensor("wkT", [P, DC, E], BF16, kind="ExternalInput").ap()
    wv_d = nc.dram_tensor("wvT", [P, DC, E], BF16, kind="ExternalInput").ap()
    wo_d = nc.dram_tensor("woT", [P, 2, DD], BF16, kind="ExternalInput").ap()
    tri_d = nc.dram_tensor("tri", [P, P], BF16, kind="ExternalInput").ap()
    out_d = nc.dram_tensor("out", [S, DD], F32, kind="ExternalOutput").ap()
    with tile.TileContext(nc) as tc:
        _emit(tc, nc, xT_d, wq_d, wk_d, wv_d, wo_d, tri_d, out_d)
    if split_waits:
        _split_multi_waits(nc)
    return nc


def _strip(a, chunks):
    """[D, N] -> [128, D//128, N] with partition-major layout, contiguous."""
    import ml_dtypes

    d, n = a.shape
    return np.ascontiguousarray(
        a.reshape(chunks, P, n).transpose(1, 0, 2).astype(ml_dtypes.bfloat16)
    )


def make_in_maps(x, Wq, Wk, Wv, Wo):
    import ml_dtypes

    tri = np.ascontiguousarray(
        np.triu(np.ones((P, P), np.float32)).astype(ml_dtypes.bfloat16)
    )
    in_maps = []
    for c in range(8):
        b, g = c // 4, c % 4
        sl = slice(E * g, E * (g + 1))
        in_maps.append(
            {
                "xT": _strip(x[b].T.astype(np.float32), DC),
                "wqT": _strip((Wq[sl, :] * 0.125).T.astype(np.float32), DC),
                "wkT": _strip(Wk[sl, :].T.astype(np.float32), DC),
                "wvT": _strip(Wv[sl, :].T.astype(np.float32), DC),
                "woT": _strip(Wo[:, sl].T.astype(np.float32), 2),
                "tri": tri,
            }
        )
    return in_maps


def kernel(x, Wq, Wk, Wv, Wo, bo, _run_kwargs=None):
    x, Wq, Wk, Wv, Wo, bo = (
        np.asarray(a, dtype=np.float32) for a in (x, Wq, Wk, Wv, Wo, bo)
    )
    nc = build_nc()
    in_maps = make_in_maps(x, Wq, Wk, Wv, Wo)
    res = run_bass_kernel_spmd(
        nc, in_maps, core_ids=list(range(8)), **(_run_kwargs or {})
    )
    out = np.zeros((2, S, DD), dtype=np.float32)
    for c in range(8):
        out[c // 4] += res.results[c]["out"]
    out += bo[None, None, :]
    if _run_kwargs:
        kernel.last_results = res
    return out
